# revision 1
# baseline (speedup 1.0000x reference)
"""Trainium2 Bass kernel for nn_BitwiseTasNet (encoder + 32 linear residual
blocks + sigmoid mask + transposed-conv decoder).

Restructuring (all folding host-side, exact in fp32):
  - eval-mode BatchNorms fold into GEMM weights / per-channel affine applied
    at PSUM eviction (ScalarE activation with per-partition scale+bias).
  - per-block additive constants propagate forward into the next block's
    eviction bias; the total lands in the final sigmoid's bias vector.
  - dilated depthwise 3-tap conv on zero-haloed SBUF tiles: D-rows 0-2 on
    VectorE (3x tensor_scalar_mul + 2x tensor_add), row 3 via 3 pre-scaled
    ScalarE evictions + 2 VectorE adds (GpSimd measured slow + DVE-port toxic).
  - residual adds ride the PE: identity-matmul preloads h into the GEMM2
    PSUM accumulation; ScalarE evicts h+r straight back to the bf16 stream.
  - bf16 residual stream (validated rel_l2 1.25e-2 vs reference).
  - encoder = host im2col (stride view) + GEMM.
  - decoder = 2 shifted GEMMs accumulating in PSUM (overlap-add on PE).

Sharding: data-parallel over batch N=4 on 4 cores (pair-collectives measured
at ~20us/shot on this stack — per-block cross-core comm is not viable).
"""
import sys
import numpy as np
import ml_dtypes

sys.path.insert(0, "/opt/trn_rl_repo")

from concourse import bass, bacc, tile, mybir  # noqa: E402
from concourse.bass_utils import run_bass_kernel_spmd  # noqa: E402

# model dims (hardcoded per contract)
N, CIN, T = 4, 1, 8000
C, D, K = 256, 512, 3
FK, FS = 20, 10
REPEATS, BLOCKS = 4, 8
NB = REPEATS * BLOCKS
EPS = 1e-5
L = 803
W4 = 804               # even op width for DVE 4x mode
PAD = 128              # t-tile halo (max dilation)
TW = PAD + W4 + PAD
CHUNKS = [(0, 512), (512, L)]   # psum-bank-aligned matmul free-dim chunks

F32 = mybir.dt.float32
BF16 = mybir.dt.bfloat16
bf16 = ml_dtypes.bfloat16
AF = mybir.ActivationFunctionType
ALU = mybir.AluOpType

NVC = 4 * 5 + 6        # vec columns per block


# ----------------------------------------------------------------- host math
def fold_params(inp):
    p = {k: np.asarray(v, dtype=np.float64) for k, v in inp.items()}
    a = {}
    for nm in ('bn1', 'bn2', 'bn3'):
        sc = p[nm + '_g'] / np.sqrt(p[nm + '_v'] + EPS)
        sh = p[nm + '_b'] - p[nm + '_m'] * sc
        a[nm] = (sc, sh)
    a1, c1 = a['bn1']; a2, c2 = a['bn2']; a3, c3 = a['bn3']
    W1p = p['w1'][:, :, :, 0] * a1[:, None, :]                 # [NB, D, C]
    beta1 = np.einsum('idc,ic->id', p['w1'][:, :, :, 0], c1)   # [NB, D]
    Wk = a3[:, None, :] * np.transpose(p['wd'][:, :, 0, :], (0, 2, 1))  # [NB,3,D]
    W2 = p['w2'][:, :, :, 0]                                   # [NB, C, D]
    beta2 = np.einsum('icd,id->ic', W2, c3)                    # [NB, C]
    s = np.zeros((NB + 1, C))
    for i in range(NB):
        s[i + 1] = s[i] + beta2[i]
    b2p = a2 * (beta1 + np.einsum('idc,ic->id', W1p, s[:NB])) + c2  # [NB, D]
    return dict(W1p=W1p, Wk=Wk, W2=W2, a2=a2, b2p=b2p, sig_bias=s[NB],
                Wenc=p['w_enc'][:, 0, :], Wdec=p['w_dec'][:, 0, :])


def im2col(x):
    xp = np.zeros((N, T + 2 * FK), dtype=np.float32)
    xp[:, FK:FK + T] = np.asarray(x, np.float32)[:, 0, :]
    idx = FS * np.arange(L)[None, :] + np.arange(FK)[:, None]  # [FK, L]
    return xp[:, idx]                                          # [N, FK, L]


def pack_host(f):
    """Pack folded params into DMA-friendly arrays."""
    w1t = np.zeros((NB, 128, 2 * D), np.float32)
    for k in range(2):
        w1t[:, :, k * D:(k + 1) * D] = np.transpose(
            f['W1p'][:, :, k * 128:(k + 1) * 128], (0, 2, 1))
    w2t = np.zeros((NB, 128, 4 * C), np.float32)
    for k in range(4):
        w2t[:, :, k * C:(k + 1) * C] = np.transpose(
            f['W2'][:, :, k * 128:(k + 1) * 128], (0, 2, 1))
    wenct = f['Wenc'].T.astype(np.float32)                     # [20, 256]
    wdect = np.zeros((128, 40), np.float32)
    for k in range(2):
        wdect[:, k * 20:(k + 1) * 20] = f['Wdec'][k * 128:(k + 1) * 128, :]
    # per-partition vectors: per block: 4x(a2,b2,W0,W1,W2) + row-3 E0..E2,F0..F2
    nv = NB * NVC + 2
    vecs = np.zeros((128, nv), np.float32)
    for i in range(NB):
        for m in range(4):
            base = i * NVC + m * 5
            sl = slice(m * 128, (m + 1) * 128)
            vecs[:, base + 0] = f['a2'][i][sl]
            vecs[:, base + 1] = f['b2p'][i][sl]
            for kk in range(3):
                vecs[:, base + 2 + kk] = f['Wk'][i, kk][sl]
        sl = slice(3 * 128, 4 * 128)
        for kk in range(3):
            vecs[:, i * NVC + 20 + kk] = (f['a2'][i] * f['Wk'][i, kk])[sl]
            vecs[:, i * NVC + 23 + kk] = (f['b2p'][i] * f['Wk'][i, kk])[sl]
    for mc in range(2):
        vecs[:, NB * NVC + mc] = f['sig_bias'][mc * 128:(mc + 1) * 128]
    return dict(
        w1t=w1t.astype(bf16), w2t=w2t.astype(bf16),
        wenct=wenct.astype(bf16), wdect=wdect.astype(bf16), vecs=vecs)


def vcol(i, m, kind):
    off = {'a2': 0, 'b2': 1, 'W0': 2, 'W1': 3, 'W2': 4}[kind]
    return i * NVC + m * 5 + off


def vcol3(i, kind):
    off = {'E0': 20, 'E1': 21, 'E2': 22, 'F0': 23, 'F1': 24, 'F2': 25}[kind]
    return i * NVC + off


# -------------------------------------------------------------- device build
def build_nc(n_cores=4, n_blocks=NB):
    nc = bacc.Bacc("TRN2", target_bir_lowering=False, debug=False,
                   num_devices=n_cores)
    xcol_d = nc.dram_tensor("xcol", [FK, L], BF16, kind="ExternalInput")
    w1_d = nc.dram_tensor("w1t", [NB, 128, 2 * D], BF16, kind="ExternalInput")
    w2_d = nc.dram_tensor("w2t", [NB, 128, 4 * C], BF16, kind="ExternalInput")
    wenc_d = nc.dram_tensor("wenct", [FK, C], BF16, kind="ExternalInput")
    wdec_d = nc.dram_tensor("wdect", [128, 40], BF16, kind="ExternalInput")
    vecs_d = nc.dram_tensor("vecs", [128, NB * NVC + 2], F32,
                            kind="ExternalInput")
    id_d = nc.dram_tensor("ident", [128, 128], BF16, kind="ExternalInput")
    out_d = nc.dram_tensor("out", [10, 800], F32, kind="ExternalOutput")

    with tile.TileContext(nc) as tc:
        with (
            tc.tile_pool(name="fix", bufs=1) as fix,
            tc.tile_pool(name="w1p", bufs=6) as w1pool,
            tc.tile_pool(name="w2p", bufs=6) as w2pool,
            tc.tile_pool(name="pg", bufs=2, space="PSUM") as pgp,
            tc.tile_pool(name="pr", bufs=4, space="PSUM") as prp,
        ):
            vecs = fix.tile([128, NB * NVC + 2], F32, tag="vecs")
            xcol = fix.tile([FK, L], BF16, tag="xcol")
            wenc = fix.tile([FK, C], BF16, tag="wenc")
            wdec = fix.tile([128, 40], BF16, tag="wdec")
            ident = fix.tile([128, 128], BF16, tag="ident")
            hb = [fix.tile([128, W4], BF16, tag=f"hb{m}", name=f"hb{m}")
                  for m in range(2)]
            xe = [fix.tile([128, L], F32, tag=f"xe{m}", name=f"xe{m}")
                  for m in range(2)]
            # rows 0-2: plain t; row 3: three pre-scaled copies
            tt = [fix.tile([128, TW], BF16, tag=f"t{m}", name=f"t{m}")
                  for m in range(3)]
            t3 = [fix.tile([128, TW], BF16, tag=f"t3{k}", name=f"t3{k}")
                  for k in range(3)]
            vv = [fix.tile([128, W4], BF16, tag=f"v{m}", name=f"v{m}")
                  for m in range(4)]
            tmp = [fix.tile([128, W4], BF16, tag=f"tmp{m}", name=f"tmp{m}")
                   for m in range(2)]
            tmp2 = fix.tile([128, W4], BF16, tag="tmp2")
            yy = [fix.tile([128, L], BF16, tag=f"y{m}", name=f"y{m}")
                  for m in range(2)]
            outsb = fix.tile([10, 800], F32, tag="outsb")

            nc.sync.dma_start(out=vecs[:], in_=vecs_d.ap())
            nc.sync.dma_start(out=xcol[:], in_=xcol_d.ap())
            nc.sync.dma_start(out=wenc[:], in_=wenc_d.ap())
            nc.sync.dma_start(out=wdec[:], in_=wdec_d.ap())
            nc.sync.dma_start(out=ident[:], in_=id_d.ap())

            # zero halos once (never written again)
            for t in tt + t3:
                nc.gpsimd.memset(t[:, 0:PAD], 0.0)
                nc.gpsimd.memset(t[:, PAD + L:TW], 0.0)

            # ---- encoder: h0 = xe = Wenc @ xcol ----
            for mc in range(2):
                pe = pgp.tile([128, L], F32, tag="pg", name="pe")
                for (c0, c1) in CHUNKS:
                    nc.tensor.matmul(
                        pe[:, c0:c1], wenc[:, mc * 128:(mc + 1) * 128],
                        xcol[:, c0:c1], start=True, stop=True)
                nc.scalar.copy(hb[mc][:, 0:L], pe[:])
                nc.vector.tensor_copy(xe[mc][:], pe[:])

            # ---- residual blocks ----
            for i in range(n_blocks):
                d = 2 ** (i % BLOCKS)
                w1 = w1pool.tile([128, 2 * D], BF16, tag="w1", name="w1")
                w2 = w2pool.tile([128, 4 * C], BF16, tag="w2", name="w2")
                nc.sync.dma_start(out=w1[:], in_=w1_d.ap()[i])
                nc.sync.dma_start(out=w2[:], in_=w2_d.ap()[i])

                # GEMM1 + eviction per D-row
                for m in (0, 1, 2, 3):
                    ps = pgp.tile([128, L], F32, tag="pg", name="ps")
                    for (c0, c1) in CHUNKS:
                        for k in range(2):
                            nc.tensor.matmul(
                                ps[:, c0:c1],
                                w1[:, k * D + m * 128: k * D + (m + 1) * 128],
                                hb[k][:, c0:c1],
                                start=(k == 0), stop=(k == 1))
                    if m < 3:
                        nc.scalar.activation(
                            tt[m][:, PAD:PAD + L], ps[:], AF.Identity,
                            bias=vecs[:, vcol(i, m, 'b2'):vcol(i, m, 'b2') + 1],
                            scale=vecs[:, vcol(i, m, 'a2'):vcol(i, m, 'a2') + 1])
                    else:
                        for kk in range(3):
                            e = vcol3(i, f'E{kk}'); f_ = vcol3(i, f'F{kk}')
                            nc.scalar.activation(
                                t3[kk][:, PAD:PAD + L], ps[:], AF.Identity,
                                bias=vecs[:, f_:f_ + 1], scale=vecs[:, e:e + 1])

                # taps rows 0-2: DVE ts-muls; rows 0,1 DVE adds, row 2 GpSimd adds
                for m in range(3):
                    c = [vecs[:, vcol(i, m, w):vcol(i, m, w) + 1]
                         for w in ('W0', 'W1', 'W2')]
                    nc.vector.tensor_scalar_mul(
                        vv[m][:], tt[m][:, PAD - d:PAD - d + W4], c[0])
                    nc.vector.tensor_scalar_mul(
                        tmp[0][:], tt[m][:, PAD:PAD + W4], c[1])
                    nc.vector.tensor_add(vv[m][:], vv[m][:], tmp[0][:])
                    nc.vector.tensor_scalar_mul(
                        tmp[1][:], tt[m][:, PAD + d:PAD + d + W4], c[2])
                    nc.vector.tensor_add(vv[m][:], vv[m][:], tmp[1][:])
                # row 3 on GpSimd (pre-scaled evictions)
                nc.vector.tensor_add(
                    vv[3][:], t3[0][:, PAD - d:PAD - d + W4],
                    t3[1][:, PAD:PAD + W4])
                nc.vector.tensor_add(
                    vv[3][:], vv[3][:], t3[2][:, PAD + d:PAD + d + W4])

                # GEMM2 + residual add (bf16 stream)
                prs = {}
                for mc in range(2):
                    for ci, (c0, c1) in enumerate(CHUNKS):
                        prs[(mc, ci)] = prp.tile([128, 512], F32, tag="pr",
                                                 name="prt")
                for ci, (c0, c1) in enumerate(CHUNKS):
                    nc.tensor.matmul(prs[(0, ci)][:, 0:c1 - c0], ident[:],
                                     hb[0][:, c0:c1], start=True, stop=False)
                for k in range(4):
                    for mc in range(2):
                        for ci, (c0, c1) in enumerate(CHUNKS):
                            nc.tensor.matmul(
                                prs[(mc, ci)][:, 0:c1 - c0],
                                w2[:, k * C + mc * 128: k * C + (mc + 1) * 128],
                                vv[k][:, c0:c1],
                                start=(k == 0 and mc == 1), stop=(k == 3))
                for ci, (c0, c1) in enumerate(CHUNKS):
                    nc.scalar.copy(hb[0][:, c0:c1], prs[(0, ci)][:, 0:c1 - c0])
                    nc.vector.tensor_add(hb[1][:, c0:c1], hb[1][:, c0:c1],
                                         prs[(1, ci)][:, 0:c1 - c0])

            # ---- mask + decoder ----
            sb = NB * NVC
            for mc in range(2):
                mask = vv[mc]
                nc.scalar.activation(mask[:, 0:L], hb[mc][:, 0:L], AF.Sigmoid,
                                     bias=vecs[:, sb + mc:sb + mc + 1])
                nc.vector.tensor_mul(yy[mc][:], xe[mc][:], mask[:, 0:L])

            for (c0, c1) in [(0, 512), (512, 800)]:
                po = prp.tile([10, 512], F32, tag="pr", name="po")
                for k in range(2):
                    nc.tensor.matmul(
                        po[:, 0:c1 - c0], wdec[:, k * 20:k * 20 + 10],
                        yy[k][:, c0 + 2:c1 + 2], start=(k == 0), stop=False)
                    nc.tensor.matmul(
                        po[:, 0:c1 - c0], wdec[:, k * 20 + 10:k * 20 + 20],
                        yy[k][:, c0 + 1:c1 + 1], start=False, stop=(k == 1))
                nc.scalar.copy(outsb[:, c0:c1], po[:, 0:c1 - c0])
            nc.sync.dma_start(out=out_d.ap(), in_=outsb[:])

    nc.compile()
    return nc


# ------------------------------------------------------------------- driver
_IDENT = np.eye(128, dtype=bf16)
_CACHE = {}


def _get_nc(n_cores, n_blocks):
    key = (n_cores, n_blocks)
    if key not in _CACHE:
        _CACHE[key] = build_nc(n_cores, n_blocks)
    return _CACHE[key]


def run(inputs, n_blocks=NB, trace=False):
    f = fold_params(inputs)
    pk = pack_host(f)
    xc = im2col(inputs['x']).astype(bf16)
    n_cores = 4
    nc = _get_nc(n_cores, n_blocks)
    in_maps = []
    for n in range(n_cores):
        in_maps.append(dict(
            xcol=xc[n], w1t=pk['w1t'], w2t=pk['w2t'], ident=_IDENT,
            wenct=pk['wenct'], wdect=pk['wdect'], vecs=pk['vecs']))
    res = run_bass_kernel_spmd(nc, in_maps, list(range(n_cores)), trace=trace)
    out = np.zeros((N, CIN, T), np.float32)
    for n in range(n_cores):
        out[n, 0, :] = res.results[n]['out'].T.reshape(T)
    return out, res


def kernel(**inputs):
    out, _ = run(inputs)
    return out



# revision 2
# speedup vs baseline: 1.0772x; 1.0772x over previous
"""Trainium2 Bass kernel for nn_BitwiseTasNet (encoder + 32 linear residual
blocks + sigmoid mask + transposed-conv decoder).

v2 restructuring (all folding host-side, exact in fp32):
  - residual stream h lives in PSUM across all 32 blocks: GEMM2 accumulates
    onto persistent hp tiles (start=False), killing the identity-matmul
    preloads and DVE residual adds, and keeping h in fp32 end-to-end.
  - eval-mode BatchNorms fold into GEMM weights / eviction affine.
  - ratio-fold taps: GEMM1 PSUM evicted once per D-row with scale W1*a2 and
    bias W1*b2p (center tap pre-applied).  The dilated 3-tap depthwise conv
    is then v = r0*tc(t-d) + tc + r2*tc(t+d) with per-channel ratios
    r0=W0/W1, r2=W2/W1: 2 DVE tensor_scalar_mul (4x mode) + 2 tensor_add
    (2x mode) per row.  Absolute error stays ~bf16 of each tap term, so the
    heavy-tailed ratios are safe (validated 8.5e-3 rel_l2 vs reference).
  - per-block additive constants propagate forward; total lands in the final
    sigmoid bias (applied straight from PSUM).
  - encoder = host im2col + GEMM into hp (opens the accumulation group).
  - decoder = 2 shifted GEMMs accumulating in PSUM (overlap-add on PE).

Sharding: data-parallel over batch N=4 on 4 cores (pair-collectives ~20us
per shot on this stack - per-block cross-core comm is not viable).
"""
import sys
import numpy as np
import ml_dtypes

sys.path.insert(0, "/opt/trn_rl_repo")

from concourse import bass, bacc, tile, mybir  # noqa: E402
from concourse.bass_utils import run_bass_kernel_spmd  # noqa: E402

# model dims (hardcoded per contract)
N, CIN, T = 4, 1, 8000
C, D, K = 256, 512, 3
FK, FS = 20, 10
REPEATS, BLOCKS = 4, 8
NB = REPEATS * BLOCKS
EPS = 1e-5
L = 803
W4 = 804               # even op width for DVE 4x mode
PAD = 128              # t-tile halo (max dilation)
TW = PAD + W4 + PAD
CHUNKS = [(0, 512), (512, L)]   # psum-bank-aligned matmul free-dim chunks

F32 = mybir.dt.float32
BF16 = mybir.dt.bfloat16
bf16 = ml_dtypes.bfloat16
AF = mybir.ActivationFunctionType
ALU = mybir.AluOpType

NVC = 16               # vec columns per block: 4 rows x (scale,bias,r0,r2)


# ----------------------------------------------------------------- host math
def fold_params(inp):
    p = {k: np.asarray(v, dtype=np.float64) for k, v in inp.items()}
    a = {}
    for nm in ('bn1', 'bn2', 'bn3'):
        sc = p[nm + '_g'] / np.sqrt(p[nm + '_v'] + EPS)
        sh = p[nm + '_b'] - p[nm + '_m'] * sc
        a[nm] = (sc, sh)
    a1, c1 = a['bn1']; a2, c2 = a['bn2']; a3, c3 = a['bn3']
    W1p = p['w1'][:, :, :, 0] * a1[:, None, :]                 # [NB, D, C]
    beta1 = np.einsum('idc,ic->id', p['w1'][:, :, :, 0], c1)   # [NB, D]
    Wk = a3[:, None, :] * np.transpose(p['wd'][:, :, 0, :], (0, 2, 1))  # [NB,3,D]
    W2 = p['w2'][:, :, :, 0]                                   # [NB, C, D]
    beta2 = np.einsum('icd,id->ic', W2, c3)                    # [NB, C]
    s = np.zeros((NB + 1, C))
    for i in range(NB):
        s[i + 1] = s[i] + beta2[i]
    b2p = a2 * (beta1 + np.einsum('idc,ic->id', W1p, s[:NB])) + c2  # [NB, D]
    return dict(W1p=W1p, Wk=Wk, W2=W2, a2=a2, b2p=b2p, sig_bias=s[NB],
                Wenc=p['w_enc'][:, 0, :], Wdec=p['w_dec'][:, 0, :])


def im2col(x):
    xp = np.zeros((N, T + 2 * FK), dtype=np.float32)
    xp[:, FK:FK + T] = np.asarray(x, np.float32)[:, 0, :]
    idx = FS * np.arange(L)[None, :] + np.arange(FK)[:, None]  # [FK, L]
    return xp[:, idx]                                          # [N, FK, L]


def pack_host(f):
    """Pack folded params into DMA-friendly arrays."""
    w1t = np.zeros((NB, 128, 2 * D), np.float32)
    for k in range(2):
        w1t[:, :, k * D:(k + 1) * D] = np.transpose(
            f['W1p'][:, :, k * 128:(k + 1) * 128], (0, 2, 1))
    w2t = np.zeros((NB, 128, 4 * C), np.float32)
    for k in range(4):
        w2t[:, :, k * C:(k + 1) * C] = np.transpose(
            f['W2'][:, :, k * 128:(k + 1) * 128], (0, 2, 1))
    wenct = f['Wenc'].T.astype(np.float32)                     # [20, 256]
    wdect = np.zeros((128, 40), np.float32)
    for k in range(2):
        wdect[:, k * 20:(k + 1) * 20] = f['Wdec'][k * 128:(k + 1) * 128, :]
    # per-partition vectors, per block per D-row: scale, bias, r0, r2
    scale_f = f['Wk'][:, 1, :] * f['a2']                       # [NB, D]
    bias_f = f['Wk'][:, 1, :] * f['b2p']
    r0 = f['Wk'][:, 0, :] / f['Wk'][:, 1, :]
    r2 = f['Wk'][:, 2, :] / f['Wk'][:, 1, :]
    nv = NB * NVC + 2
    vecs = np.zeros((128, nv), np.float32)
    for i in range(NB):
        for m in range(4):
            base = i * NVC + m * 4
            sl = slice(m * 128, (m + 1) * 128)
            vecs[:, base + 0] = scale_f[i][sl]
            vecs[:, base + 1] = bias_f[i][sl]
            vecs[:, base + 2] = r0[i][sl]
            vecs[:, base + 3] = r2[i][sl]
    for mc in range(2):
        vecs[:, NB * NVC + mc] = f['sig_bias'][mc * 128:(mc + 1) * 128]
    return dict(
        w1t=w1t.astype(bf16), w2t=w2t.astype(bf16),
        wenct=wenct.astype(bf16), wdect=wdect.astype(bf16), vecs=vecs)


def vcol(i, m, kind):
    off = {'sc': 0, 'bi': 1, 'r0': 2, 'r2': 3}[kind]
    return i * NVC + m * 4 + off


# -------------------------------------------------------------- device build
def build_nc(n_cores=4, n_blocks=NB):
    nc = bacc.Bacc("TRN2", target_bir_lowering=False, debug=False,
                   num_devices=n_cores)
    xcol_d = nc.dram_tensor("xcol", [FK, L], BF16, kind="ExternalInput")
    w1_d = nc.dram_tensor("w1t", [NB, 128, 2 * D], BF16, kind="ExternalInput")
    w2_d = nc.dram_tensor("w2t", [NB, 128, 4 * C], BF16, kind="ExternalInput")
    wenc_d = nc.dram_tensor("wenct", [FK, C], BF16, kind="ExternalInput")
    wdec_d = nc.dram_tensor("wdect", [128, 40], BF16, kind="ExternalInput")
    vecs_d = nc.dram_tensor("vecs", [128, NB * NVC + 2], F32,
                            kind="ExternalInput")
    out_d = nc.dram_tensor("out", [10, 800], F32, kind="ExternalOutput")

    with tile.TileContext(nc) as tc:
        with (
            tc.tile_pool(name="fix", bufs=1) as fix,
            tc.tile_pool(name="w1p", bufs=6) as w1pool,
            tc.tile_pool(name="w2p", bufs=6) as w2pool,
            tc.tile_pool(name="hps", bufs=1, space="PSUM") as hps,
            tc.tile_pool(name="pg", bufs=2, space="PSUM") as pgp,
        ):
            vecs = fix.tile([128, NB * NVC + 2], F32, tag="vecs")
            xcol = fix.tile([FK, L], BF16, tag="xcol")
            wenc = fix.tile([FK, C], BF16, tag="wenc")
            wdec = fix.tile([128, 40], BF16, tag="wdec")
            hb = [fix.tile([128, L], BF16, tag=f"hb{m}", name=f"hb{m}")
                  for m in range(2)]
            xe = [fix.tile([128, L], BF16, tag=f"xe{m}", name=f"xe{m}")
                  for m in range(2)]
            tt = [fix.tile([128, TW], BF16, tag=f"t{m}", name=f"t{m}")
                  for m in range(4)]
            vv = [fix.tile([128, W4], BF16, tag=f"v{m}", name=f"v{m}")
                  for m in range(4)]
            tmp = [fix.tile([128, W4], BF16, tag=f"tmp{m}", name=f"tmp{m}")
                   for m in range(4)]
            yy = [fix.tile([128, L], BF16, tag=f"y{m}", name=f"y{m}")
                  for m in range(2)]
            outsb = fix.tile([10, 800], F32, tag="outsb")
            # persistent residual stream in PSUM
            hp = [hps.tile([128, L], F32, tag=f"hp{m}", name=f"hp{m}")
                  for m in range(2)]

            nc.sync.dma_start(out=vecs[:], in_=vecs_d.ap())
            nc.sync.dma_start(out=xcol[:], in_=xcol_d.ap())
            nc.sync.dma_start(out=wenc[:], in_=wenc_d.ap())
            nc.sync.dma_start(out=wdec[:], in_=wdec_d.ap())

            # zero halos once (never written again)
            for t in tt:
                nc.gpsimd.memset(t[:, 0:PAD], 0.0)
                nc.gpsimd.memset(t[:, PAD + L:TW], 0.0)

            # ---- encoder: hp = Wenc @ xcol (opens the h accumulation) ----
            for mc in range(2):
                for (c0, c1) in CHUNKS:
                    nc.tensor.matmul(
                        hp[mc][:, c0:c1], wenc[:, mc * 128:(mc + 1) * 128],
                        xcol[:, c0:c1], start=True, stop=False)
                nc.scalar.copy(hb[mc][:], hp[mc][:])
                nc.vector.tensor_copy(xe[mc][:], hp[mc][:])

            # ---- residual blocks ----
            for i in range(n_blocks):
                d = 2 ** (i % BLOCKS)
                last = (i == n_blocks - 1)
                w1 = w1pool.tile([128, 2 * D], BF16, tag="w1", name="w1")
                w2 = w2pool.tile([128, 4 * C], BF16, tag="w2", name="w2")
                nc.sync.dma_start(out=w1[:], in_=w1_d.ap()[i])
                nc.sync.dma_start(out=w2[:], in_=w2_d.ap()[i])

                # GEMM1 + affine eviction + ratio-fold taps, per D-row
                for m in range(4):
                    ps = pgp.tile([128, L], F32, tag="pg", name="ps")
                    for k in range(2):
                        for (c0, c1) in CHUNKS:
                            nc.tensor.matmul(
                                ps[:, c0:c1],
                                w1[:, k * D + m * 128: k * D + (m + 1) * 128],
                                hb[k][:, c0:c1],
                                start=(k == 0), stop=(k == 1))
                    nc.scalar.activation(
                        tt[m][:, PAD:PAD + L], ps[:], AF.Identity,
                        bias=vecs[:, vcol(i, m, 'bi'):vcol(i, m, 'bi') + 1],
                        scale=vecs[:, vcol(i, m, 'sc'):vcol(i, m, 'sc') + 1])
                    nc.vector.tensor_scalar_mul(
                        tmp[m][:], tt[m][:, PAD - d:PAD - d + W4],
                        vecs[:, vcol(i, m, 'r0'):vcol(i, m, 'r0') + 1])
                    nc.vector.tensor_add(
                        vv[m][:], tmp[m][:], tt[m][:, PAD:PAD + W4])
                    nc.vector.tensor_scalar_mul(
                        tmp[m][:], tt[m][:, PAD + d:PAD + d + W4],
                        vecs[:, vcol(i, m, 'r2'):vcol(i, m, 'r2') + 1])
                    nc.vector.tensor_add(vv[m][:], vv[m][:], tmp[m][:])

                # GEMM2 accumulates straight onto the fp32 residual stream
                for k in range(4):
                    for mc in range(2):
                        for (c0, c1) in CHUNKS:
                            nc.tensor.matmul(
                                hp[mc][:, c0:c1],
                                w2[:, k * C + mc * 128: k * C + (mc + 1) * 128],
                                vv[k][:, c0:c1],
                                start=False, stop=(last and k == 3))
                if not last:
                    nc.scalar.copy(hb[0][:], hp[0][:])
                    nc.vector.tensor_copy(hb[1][:], hp[1][:])

            # ---- mask + decoder ----
            sb = NB * NVC
            for mc in range(2):
                mask = tmp[mc]
                nc.scalar.activation(mask[:, 0:L], hp[mc][:], AF.Sigmoid,
                                     bias=vecs[:, sb + mc:sb + mc + 1])
                nc.vector.tensor_mul(yy[mc][:], xe[mc][:], mask[:, 0:L])

            for (c0, c1) in [(0, 512), (512, 800)]:
                po = pgp.tile([10, 512], F32, tag="pg", name="po")
                for k in range(2):
                    nc.tensor.matmul(
                        po[:, 0:c1 - c0], wdec[:, k * 20:k * 20 + 10],
                        yy[k][:, c0 + 2:c1 + 2], start=(k == 0), stop=False)
                    nc.tensor.matmul(
                        po[:, 0:c1 - c0], wdec[:, k * 20 + 10:k * 20 + 20],
                        yy[k][:, c0 + 1:c1 + 1], start=False, stop=(k == 1))
                nc.scalar.copy(outsb[:, c0:c1], po[:, 0:c1 - c0])
            nc.sync.dma_start(out=out_d.ap(), in_=outsb[:])

    nc.compile()
    return nc


# ------------------------------------------------------------------- driver
_CACHE = {}


def _get_nc(n_cores, n_blocks):
    key = (n_cores, n_blocks)
    if key not in _CACHE:
        _CACHE[key] = build_nc(n_cores, n_blocks)
    return _CACHE[key]


def run(inputs, n_blocks=NB, trace=False):
    f = fold_params(inputs)
    pk = pack_host(f)
    xc = im2col(inputs['x']).astype(bf16)
    n_cores = 4
    nc = _get_nc(n_cores, n_blocks)
    in_maps = []
    for n in range(n_cores):
        in_maps.append(dict(
            xcol=xc[n], w1t=pk['w1t'], w2t=pk['w2t'],
            wenct=pk['wenct'], wdect=pk['wdect'], vecs=pk['vecs']))
    res = run_bass_kernel_spmd(nc, in_maps, list(range(n_cores)), trace=trace)
    out = np.zeros((N, CIN, T), np.float32)
    for n in range(n_cores):
        out[n, 0, :] = res.results[n]['out'].T.reshape(T)
    return out, res


def kernel(**inputs):
    out, _ = run(inputs)
    return out
